# revision 1
# baseline (speedup 1.0000x reference)
"""Bass/Tile kernel for nn_SpaceTransformer_nat: one frame per NeuronCore.

Home layout: channel-partition [C=128, tok=2304] (matches HBM stride of x).
Attention: transposed-scores-by-key-row, 2-head-stacked contractions,
multiplicative exp(rpb)+mask tables, ones-column denominators, sliding
PSUM accumulation of O^T, deferred per-head normalization via
DMA-partition-broadcast reciprocals.
"""
from contextlib import ExitStack

import numpy as np
import ml_dtypes

import concourse.bass as bass
from concourse import mybir
from concourse.masks import make_identity

F32 = mybir.dt.float32
BF16 = mybir.dt.bfloat16
AF = mybir.ActivationFunctionType
ALU = mybir.AluOpType

K = 7
NH = 4
H = W = 48
C = 128
NTOK = H * W
NT = NTOK // 128
HID = 256
QN_MAX = 10          # max query rows per key row
NQ_MAX = QN_MAX * W  # 480


def nbr_start(h):
    return min(max(h - K // 2, 0), H - K)


def qwin(r):
    rows = [h for h in range(H) if nbr_start(h) <= r < nbr_start(h) + K]
    assert rows == list(range(rows[0], rows[0] + len(rows)))
    assert len(rows) <= QN_MAX
    return rows[0], len(rows)


def et_cls(r):
    """Dedupe class for key-row r's ET table (15 classes)."""
    if r <= 6:
        return r
    if r <= 40:
        return 7
    return r - 33


N_CLS = 15


# --------------------------------------------------------------------------
# host-side preparation
# --------------------------------------------------------------------------

def build_et_table(rpb):
    """ET [96, 2*N_CLS*NQ_MAX] bf16: for class c, pair p:
    ET[m*48+kw, (p*N_CLS+c)*NQ_MAX + dq*48 + w] = exp(rpb[2p+m, bh, bw])
    masked to the valid (h, r, w, kw) band."""
    ws = np.array([nbr_start(w) for w in range(W)])
    kw_g = np.arange(W)
    wvalid = (kw_g[:, None] >= ws[None, :]) & (kw_g[:, None] < ws[None, :] + K)
    bw_idx = np.clip(kw_g[:, None] - np.arange(W)[None, :] + (K - 1),
                     0, 2 * K - 2)

    reps = {}
    for r in range(H):
        c = et_cls(r)
        if c not in reps:
            reps[c] = r
    ET = np.zeros((96, 2 * N_CLS * NQ_MAX), np.float32)
    for c, r in reps.items():
        qs, qn = qwin(r)
        for dq in range(qn):
            h = qs + dq
            bh = r - h + (K - 1)
            assert 0 <= bh <= 2 * K - 2
            for n in range(NH):
                p, m = divmod(n, 2)
                tab = np.where(wvalid, np.exp(rpb[n, bh])[bw_idx.ravel()]
                               .reshape(W, W), 0.0)
                ET[m * 48:m * 48 + 48,
                   (p * N_CLS + c) * NQ_MAX + dq * 48:
                   (p * N_CLS + c) * NQ_MAX + dq * 48 + 48] = tab
    return ET


def prep_weights(inputs):
    """Fold LN affines + q-scale into weights; pack for the kernel."""
    bf = ml_dtypes.bfloat16
    n1w, n1b = inputs["norm1_w"], inputs["norm1_b"]
    n2w, n2b = inputs["norm2_w"], inputs["norm2_b"]
    qkv_w, qkv_b = inputs["qkv_w"], inputs["qkv_b"]
    sc = (C // NH) ** -0.5

    Wqkv = qkv_w * n1w[None, :]
    bq = qkv_w @ n1b + qkv_b
    Wqkv[0:C] *= sc
    bq0 = bq.copy()
    bq0[0:C] *= sc

    W1 = inputs["fc1_w"] * n2w[None, :]
    b1 = inputs["fc1_w"] @ n2b + inputs["fc1_b"]

    d = {
        "wqkv_t": np.ascontiguousarray(Wqkv.T).astype(bf),       # [128, 384]
        "bqkv": np.ascontiguousarray(bq0.reshape(3, 128).T).astype(np.float32),  # [128, 3]
        "wproj_t": np.ascontiguousarray(inputs["proj_w"].T).astype(bf),
        "bproj": inputs["proj_b"].reshape(-1, 1).astype(np.float32),
        "wfc1_t": np.ascontiguousarray(W1.T).astype(bf),         # [128, 256]
        "bfc1": np.ascontiguousarray(b1.reshape(2, 128).T).astype(np.float32),
        # fc2: [128, 256] rows=khalf-cols packed: wfc2_p[:, j*128:...] = fc2_w.T[128j:128j+128, :]
        "wfc2_p": np.ascontiguousarray(
            np.concatenate([inputs["fc2_w"].T[0:128, :],
                            inputs["fc2_w"].T[128:256, :]], axis=1)).astype(bf),
        "bfc2": inputs["fc2_b"].reshape(-1, 1).astype(np.float32),
        "et": build_et_table(inputs["rpb"]).astype(bf),
    }
    return d


# --------------------------------------------------------------------------
# kernel emission
# --------------------------------------------------------------------------

def ln_block(nc, tc, sb, src_cp, dst_cp, ident, eps, tag):
    """Per-token LN over channels (gamma/beta folded downstream).
    src_cp, dst_cp: [128, NTOK] bf16 SBUF. Phase 1: transpose + stats;
    phase 2: one batched sqrt/recip; phase 3: apply + transpose back."""
    with tc.tile_pool(name=f"lnps{tag}", bufs=2, space="PSUM") as ps:
        xTs = sb.tile([128, NT * 128], BF16, tag=f"lnxT{tag}")
        mvall = sb.tile([128, NT, 2], F32, tag=f"lnmv{tag}")
        for it in range(NT):
            s = slice(it * 128, (it + 1) * 128)
            xT_ps = ps.tile([128, 128], BF16, tag=f"lnTp{tag}")
            nc.tensor.transpose(xT_ps[:], src_cp[:, s], ident[:])
            nc.vector.tensor_copy(xTs[:, s], xT_ps[:])
            stats = sb.tile([128, 6], F32, tag=f"lnst{tag}")
            nc.vector.bn_stats(stats[:], xTs[:, s])
            nc.vector.bn_aggr(mvall[:, it, :], stats[:])
        # batched rstd = 1/sqrt(var + eps)
        stdall = sb.tile([128, NT], F32, tag=f"lnsd{tag}")
        nc.scalar.activation(stdall[:], mvall[:, :, 1], AF.Sqrt, bias=eps[:])
        rstdall = sb.tile([128, NT], F32, tag=f"lnrs{tag}")
        nc.vector.reciprocal(rstdall[:], stdall[:])
        for it in range(NT):
            s = slice(it * 128, (it + 1) * 128)
            uT = sb.tile([128, 128], BF16, tag=f"lnuT{tag}")
            nc.vector.tensor_scalar(
                out=uT[:], in0=xTs[:, s], scalar1=mvall[:, it, 0:1],
                scalar2=rstdall[:, it:it + 1],
                op0=ALU.subtract, op1=ALU.mult)
            u_ps = ps.tile([128, 128], BF16, tag=f"lnup{tag}")
            nc.tensor.transpose(u_ps[:], uT[:], ident[:])
            nc.vector.tensor_copy(dst_cp[:, s], u_ps[:])


def emit_frame(nc, tc, ctx: ExitStack, debug_taps=False):
    # ---- IO ----
    x_in = nc.declare_dram_parameter("x_frame", [C, NTOK], F32, isOutput=False)
    wqkv = nc.declare_dram_parameter("wqkv_t", [C, 3 * C], BF16, isOutput=False)
    bqkv = nc.declare_dram_parameter("bqkv", [C, 3], F32, isOutput=False)
    wproj = nc.declare_dram_parameter("wproj_t", [C, C], BF16, isOutput=False)
    bproj = nc.declare_dram_parameter("bproj", [C, 1], F32, isOutput=False)
    wfc1 = nc.declare_dram_parameter("wfc1_t", [C, HID], BF16, isOutput=False)
    bfc1 = nc.declare_dram_parameter("bfc1", [C, 2], F32, isOutput=False)
    wfc2 = nc.declare_dram_parameter("wfc2_p", [C, HID], BF16, isOutput=False)
    bfc2 = nc.declare_dram_parameter("bfc2", [C, 1], F32, isOutput=False)
    et_in = nc.declare_dram_parameter("et", [96, 2 * N_CLS * NQ_MAX], BF16,
                                      isOutput=False)
    out_d = nc.declare_dram_parameter("out_frame", [C, NTOK], F32,
                                      isOutput=True)
    taps = {}
    if debug_taps:
        for nm, shp, dt in [("tap_u", [C, NTOK], BF16),
                            ("tap_qkv", [C, 3 * NTOK], BF16),
                            ("tap_k2h", [C, H * 96], BF16),
                            ("tap_vT2h", [96, H * 2 * 98], BF16),
                            ("tap_osb", [C, NTOK], BF16),
                            ("tap_dent", [C, NTOK], F32),
                            ("tap_onorm", [C, NTOK], BF16),
                            ("tap_t", [C, NTOK], F32),
                            ("tap_z", [C, NTOK], BF16),
                            ("tap_g", [C, 2 * NTOK], BF16)]:
            taps[nm] = nc.declare_dram_parameter(nm, shp, dt, isOutput=True)

    sb = ctx.enter_context(tc.tile_pool(name="sb", bufs=3))
    big = ctx.enter_context(tc.tile_pool(name="big", bufs=1))

    # ---- loads ----
    x = big.tile([C, NTOK], F32)
    nc.sync.dma_start(x[:], x_in[:])
    w_qkv = big.tile([C, 3 * C], BF16)
    nc.sync.dma_start(w_qkv[:], wqkv[:])
    b_qkv = big.tile([C, 3], F32)
    nc.sync.dma_start(b_qkv[:], bqkv[:])
    w_proj = big.tile([C, C], BF16)
    nc.sync.dma_start(w_proj[:], wproj[:])
    b_proj = big.tile([C, 1], F32)
    nc.sync.dma_start(b_proj[:], bproj[:])
    w_fc1 = big.tile([C, HID], BF16)
    nc.sync.dma_start(w_fc1[:], wfc1[:])
    b_fc1 = big.tile([C, 2], F32)
    nc.sync.dma_start(b_fc1[:], bfc1[:])
    w_fc2 = big.tile([C, HID], BF16)
    nc.sync.dma_start(w_fc2[:], wfc2[:])
    b_fc2 = big.tile([C, 1], F32)
    nc.sync.dma_start(b_fc2[:], bfc2[:])
    et = big.tile([96, 2 * N_CLS * NQ_MAX], BF16)
    nc.sync.dma_start(et[:], et_in[:])

    ident = big.tile([128, 128], BF16)
    make_identity(nc, ident[:])
    eps_t = big.tile([128, 1], F32)
    nc.vector.memset(eps_t[:], 1e-5)
    zeros1 = big.tile([1, 512], BF16)
    nc.vector.memset(zeros1[:], 0.0)

    xb = big.tile([C, NTOK], BF16, tag="xb")
    nc.vector.tensor_copy(xb[:], x[:])

    # ---- LN1 + QKV ----
    u = big.tile([C, NTOK], BF16, tag="u")
    ln_block(nc, tc, sb, xb, u, ident, eps_t, "1")

    qkv = big.tile([128, 3 * NTOK], BF16)  # cols: [q | k | v] per NTOK
    with tc.tile_pool(name="qkvps", bufs=3, space="PSUM") as mm:
      for s in range(3):
        for ic in range(NTOK // 512 + (1 if NTOK % 512 else 0)):
            c0 = ic * 512
            cw = min(512, NTOK - c0)
            pt = mm.tile([128, 512], F32, tag="mmps")
            nc.tensor.matmul(pt[:, :cw], w_qkv[:, s * C:(s + 1) * C],
                             u[:, c0:c0 + cw], start=True, stop=True)
            nc.vector.tensor_scalar_add(
                out=qkv[:, s * NTOK + c0: s * NTOK + c0 + cw],
                in0=pt[:, :cw], scalar1=b_qkv[:, s:s + 1])

    q_cp = qkv[:, 0:NTOK]
    k_cp = qkv[:, NTOK:2 * NTOK]
    v_cp = qkv[:, 2 * NTOK:3 * NTOK]

    # ---- k2h: [128=(p,m,c), H*96] ----
    k2h = big.tile([128, H * 96], BF16, tag="k2h")
    nc.vector.memset(k2h[:], 0.0)
    for p in range(2):
        for m in range(2):
            rows = slice(p * 64 + m * 32, p * 64 + m * 32 + 32)
            dst = k2h[rows, :].rearrange("c (r g) -> c r g", g=96)[
                :, :, m * 48:m * 48 + W]
            src = k_cp[rows, :].rearrange("c (r w) -> c r w", r=H)
            nc.vector.tensor_copy(dst, src)

    # ---- v2h_pre: [128 rows: 66*p + (m*32..)|64,65-ones, H*96] ----
    v2h_pre = big.tile([128, 2 * H * 96], BF16, tag="v2h")   # pair p at cols p*H*96
    nc.vector.memset(v2h_pre[:], 0.0)
    for p in range(2):
        po = p * H * 96
        for m in range(2):
            rows = slice(m * 32, m * 32 + 32)
            dst = v2h_pre[rows, po:po + H * 96].rearrange(
                "c (r g) -> c r g", g=96)[:, :, m * 48:m * 48 + W]
            src = v_cp[p * 64 + m * 32:p * 64 + m * 32 + 32, :] \
                .rearrange("c (r w) -> c r w", r=H)
            nc.vector.tensor_copy(dst, src)
            ones_dst = v2h_pre[64 + 32 * m: 65 + 32 * m, po:po + H * 96] \
                .rearrange("c (r g) -> c r g", g=96)[:, :, m * 48:m * 48 + W]
            nc.vector.memset(ones_dst, 1.0)

    # transpose to vT2h [96, (r, p, 66)] -- use padded [128,96] inputs
    vT2h = big.tile([96, H * 2 * 98], BF16, tag="vT2h")
    with tc.tile_pool(name="vTpool", bufs=2, space="PSUM") as psv:
        for r in range(H):
          for p in range(2):
            tps = psv.tile([96, 128], BF16, tag="vTps")
            nc.tensor.transpose(
                tps[:], v2h_pre[:, p * H * 96 + r * 96:p * H * 96 + (r + 1) * 96],
                ident[:])
            nc.scalar.copy(
                vT2h[:, (r * 2 + p) * 98:(r * 2 + p) * 98 + 98],
                tps[:, 0:98])


    if debug_taps:
        nc.sync.dma_start(taps["tap_k2h"][:], k2h[:])
        nc.sync.dma_start(taps["tap_vT2h"][:], vT2h[:])
    # ---- attention ----
    osb = big.tile([C, NTOK], BF16)     # unnormalized O rows (c-order)
    den_t = big.tile([C, NTOK], F32, tag="dent")    # head denom at row 32*n
    nc.vector.memset(den_t[:], 1.0)
    with tc.tile_pool(name="attps", bufs=2, space="PSUM") as ps, \
         tc.tile_pool(name="attpsO", bufs=1, space="PSUM") as psO:
        for p in range(2):
            ot = psO.tile([98, NTOK], F32, tag="ot")
            # claim + zero all banks of ot via K=1 zero matmuls
            for j in range(0, NTOK, 512):
                jw = min(512, NTOK - j)
                nc.tensor.matmul(ot[:, j:j + jw], zeros1[0:1, 0:98],
                                 zeros1[0:1, 0:jw], start=True, stop=True,
                                 skip_group_check=True)
            for r in range(H):
                qs, qn = qwin(r)
                nq = 48 * qn
                cls = et_cls(r)
                st = ps.tile([96, NQ_MAX], F32, tag="st")
                nc.tensor.matmul(
                    st[:, :nq],
                    k2h[p * 64:(p + 1) * 64, r * 96:(r + 1) * 96],
                    q_cp[p * 64:(p + 1) * 64, 48 * qs:48 * qs + nq],
                    start=True, stop=True)
                pexp = sb.tile([96, NQ_MAX], BF16, tag="pexp")
                nc.scalar.activation(pexp[:, :nq], st[:, :nq], AF.Exp)
                pmul = sb.tile([96, NQ_MAX], BF16, tag="pmul")
                nc.vector.tensor_mul(
                    pmul[:, :nq], pexp[:, :nq],
                    et[:, (p * N_CLS + cls) * NQ_MAX:
                          (p * N_CLS + cls) * NQ_MAX + nq])
                # AV accumulate, split at PSUM bank boundaries (512 f32)
                c0 = 48 * qs
                b0 = (c0 // 512 + 1) * 512
                if b0 < c0 + nq:
                    splits = [(c0, b0 - c0), (b0, c0 + nq - b0)]
                else:
                    splits = [(c0, nq)]
                for (cs_, nw) in splits:
                    nc.tensor.matmul(
                        ot[:, cs_:cs_ + nw],
                        vT2h[:, (r * 2 + p) * 98:(r * 2 + p) * 98 + 98],
                        pmul[:, cs_ - c0:cs_ - c0 + nw],
                        start=False, stop=(r == H - 1), skip_group_check=True)
            # drains: c rows + denominator rows (64 -> m0, 96 -> m1)
            nc.vector.tensor_copy(osb[64 * p:64 * p + 64, :], ot[0:64, :])
            nc.vector.tensor_copy(den_t[64 * p:64 * p + 1, :], ot[64:65, :])
            nc.scalar.copy(den_t[64 * p + 32:64 * p + 33, :], ot[96:97, :])

    if debug_taps:
        nc.sync.dma_start(taps["tap_osb"][:], osb[:])
        nc.sync.dma_start(taps["tap_dent"][:], den_t[:])
    # reciprocal of denominators (valid rows 0,32,64,96; rest = 1.0)
    recip = big.tile([C, NTOK], F32, tag="v2h")
    rscr = big.tile([C, NTOK], F32, tag="vT2h")
    nc.vector.reciprocal_approx_accurate(recip[:], den_t[:], rscr[:])
    recip_b = big.tile([C, NTOK], BF16)
    nc.vector.tensor_copy(recip_b[:], recip[:])
    recip_d = nc.dram_tensor("recip_scratch", [4, NTOK], BF16)
    for n in range(4):
        nc.sync.dma_start(recip_d[n:n + 1, :],
                          recip_b[32 * n:32 * n + 1, :])
    recip_f = big.tile([C, NTOK], BF16)
    for n in range(4):
        row = recip_d[n:n + 1, :]
        src = bass.AP(tensor=row.tensor, offset=row.offset,
                      ap=[[0, 32]] + [list(xx) for xx in row.ap[1:]])
        nc.sync.dma_start(recip_f[32 * n:32 * n + 32, :], src)
    onorm = big.tile([C, NTOK], BF16)
    nc.vector.tensor_mul(onorm[:], osb[:], recip_f[:])

    if debug_taps:
        nc.sync.dma_start(taps["tap_onorm"][:], onorm[:])
    # ---- proj + residual -> t (f32), tb (bf16) ----
    mm = ctx.enter_context(tc.tile_pool(name="tailps", bufs=3, space="PSUM"))
    t_res = big.tile([C, NTOK], F32)
    for ic in range(0, NTOK, 512):
        cw = min(512, NTOK - ic)
        pt = mm.tile([128, 512], F32, tag="mmps")
        nc.tensor.matmul(pt[:, :cw], w_proj[:], onorm[:, ic:ic + cw],
                         start=True, stop=True)
        nc.vector.scalar_tensor_tensor(
            out=t_res[:, ic:ic + cw], in0=pt[:, :cw],
            scalar=b_proj[:, 0:1], in1=x[:, ic:ic + cw],
            op0=ALU.add, op1=ALU.add)
    tb = big.tile([C, NTOK], BF16, tag="xb")
    nc.vector.tensor_copy(tb[:], t_res[:])

    if debug_taps:
        nc.sync.dma_start(taps["tap_t"][:], t_res[:])
    # ---- LN2 ----
    z = big.tile([C, NTOK], BF16, tag="u")
    ln_block(nc, tc, sb, tb, z, ident, eps_t, "2")

    # ---- MLP ----
    g = big.tile([128, 2 * NTOK], BF16, tag="k2h")   # reuses k2h slot
    for j in range(2):
        for ic in range(0, NTOK, 512):
            cw = min(512, NTOK - ic)
            pt = mm.tile([128, 512], F32, tag="mmps")
            nc.tensor.matmul(pt[:, :cw], w_fc1[:, j * 128:(j + 1) * 128],
                             z[:, ic:ic + cw], start=True, stop=True)
            # gelu(a) ~ 0.5 a (1 + tanh(0.79788456 (a + 0.044715 a^3)))
            a = sb.tile([128, 512], F32, tag="ga")
            nc.vector.tensor_scalar_add(out=a[:, :cw], in0=pt[:, :cw],
                                        scalar1=b_fc1[:, j:j + 1])
            sq = sb.tile([128, 512], F32, tag="gsq")
            nc.vector.tensor_mul(sq[:, :cw], a[:, :cw], a[:, :cw])
            nc.vector.tensor_mul(sq[:, :cw], sq[:, :cw], a[:, :cw])
            nc.vector.scalar_tensor_tensor(
                out=sq[:, :cw], in0=sq[:, :cw], scalar=0.044715,
                in1=a[:, :cw], op0=ALU.mult, op1=ALU.add)
            th = sb.tile([128, 512], F32, tag="gth")
            nc.scalar.activation(th[:, :cw], sq[:, :cw], AF.Tanh,
                                 scale=0.7978845608028654)
            nc.vector.scalar_tensor_tensor(
                out=th[:, :cw], in0=th[:, :cw], scalar=1.0,
                in1=a[:, :cw], op0=ALU.add, op1=ALU.mult)
            nc.scalar.activation(
                g[:, j * NTOK + ic: j * NTOK + ic + cw], th[:, :cw],
                AF.Copy, scale=0.5)

    if debug_taps:
        nc.sync.dma_start(taps["tap_z"][:], z[:])
        nc.sync.dma_start(taps["tap_g"][:], g[:])
    out_t = big.tile([C, NTOK], F32, tag="dent")
    for ic in range(0, NTOK, 512):
        cw = min(512, NTOK - ic)
        pt = mm.tile([128, 512], F32, tag="mmps")
        for j in range(2):
            nc.tensor.matmul(pt[:, :cw], w_fc2[:, j * C:(j + 1) * C],
                             g[:, j * NTOK + ic: j * NTOK + ic + cw],
                             start=(j == 0), stop=(j == 1))
        nc.vector.scalar_tensor_tensor(
            out=out_t[:, ic:ic + cw], in0=pt[:, :cw],
            scalar=b_fc2[:, 0:1], in1=t_res[:, ic:ic + cw],
            op0=ALU.add, op1=ALU.add)

    nc.sync.dma_start(out_d[:], out_t[:])


# --------------------------------------------------------------------------
# SPMD entry point: full inputs -> full output on 8 NeuronCores
# --------------------------------------------------------------------------
import concourse.tile as _tile
import concourse.bacc as _bacc
from concourse.bass_utils import run_bass_kernel_spmd as _run_spmd

_CACHE = {}


def _get_nc():
    if "nc" not in _CACHE:
        nc = _bacc.Bacc("TRN2", target_bir_lowering=False, debug=False,
                        num_devices=8)
        with _tile.TileContext(nc) as tc:
            with ExitStack() as ctx:
                emit_frame(nc, tc, ctx)
        nc.compile()
        _CACHE["nc"] = nc
    return _CACHE["nc"]


def kernel(**inputs):
    inputs = {k: np.asarray(v) for k, v in inputs.items()}
    x = inputs["x"]
    B, Cc, D, Hh, Ww = x.shape          # (2, 128, 4, 48, 48)
    assert (B, Cc, D, Hh, Ww) == (2, 128, 4, 48, 48)
    wd = prep_weights(inputs)
    nc = _get_nc()

    in_maps = []
    for core in range(8):
        b, dd = divmod(core, D)
        frame = np.ascontiguousarray(x[b, :, dd]).reshape(C, NTOK)
        m = {"x_frame": frame.astype(np.float32)}
        m.update(wd)
        in_maps.append(m)

    res = _run_spmd(nc, in_maps, list(range(8)))
    out = np.empty((B, Cc, D, Hh, Ww), np.float32)
    for core in range(8):
        b, dd = divmod(core, D)
        out[b, :, dd] = res.results[core]["out_frame"].reshape(C, Hh, Ww)
    return out



# revision 10
# speedup vs baseline: 1.2927x; 1.2927x over previous
"""Bass/Tile kernel for nn_SpaceTransformer_nat: one frame per NeuronCore.

Home layout: channel-partition [C=128, tok=2304] (matches HBM stride of x).
Attention: transposed-scores-by-key-row, 2-head-stacked contractions,
multiplicative exp(rpb)+mask tables, ones-column denominators, sliding
PSUM accumulation of O^T, deferred per-head normalization via
DMA-partition-broadcast reciprocals.
"""
from contextlib import ExitStack

import numpy as np
import ml_dtypes

import concourse.bass as bass
from concourse import mybir
from concourse.masks import make_identity

F32 = mybir.dt.float32
BF16 = mybir.dt.bfloat16
AF = mybir.ActivationFunctionType
ALU = mybir.AluOpType

K = 7
NH = 4
H = W = 48
C = 128
NTOK = H * W
NT = NTOK // 128
HID = 256
QN_MAX = 10          # max query rows per key row
NQ_MAX = QN_MAX * W  # 480


def nbr_start(h):
    return min(max(h - K // 2, 0), H - K)


def qwin(r):
    rows = [h for h in range(H) if nbr_start(h) <= r < nbr_start(h) + K]
    assert rows == list(range(rows[0], rows[0] + len(rows)))
    assert len(rows) <= QN_MAX
    return rows[0], len(rows)


def et_cls(r):
    """Dedupe class for key-row r's ET table (15 classes)."""
    if r <= 6:
        return r
    if r <= 40:
        return 7
    return r - 33


N_CLS = 15


# --------------------------------------------------------------------------
# host-side preparation
# --------------------------------------------------------------------------

def build_et_table(rpb):
    """ET [96, 2*N_CLS*NQ_MAX] bf16: for class c, pair p:
    ET[m*48+kw, (p*N_CLS+c)*NQ_MAX + dq*48 + w] = exp(rpb[2p+m, bh, bw])
    masked to the valid (h, r, w, kw) band."""
    ws = np.array([nbr_start(w) for w in range(W)])
    kw_g = np.arange(W)
    wvalid = (kw_g[:, None] >= ws[None, :]) & (kw_g[:, None] < ws[None, :] + K)
    bw_idx = np.clip(kw_g[:, None] - np.arange(W)[None, :] + (K - 1),
                     0, 2 * K - 2)

    reps = {}
    for r in range(H):
        c = et_cls(r)
        if c not in reps:
            reps[c] = r
    ET = np.zeros((96, 2 * N_CLS * NQ_MAX), np.float32)
    for c, r in reps.items():
        qs, qn = qwin(r)
        for dq in range(qn):
            h = qs + dq
            bh = r - h + (K - 1)
            assert 0 <= bh <= 2 * K - 2
            for n in range(NH):
                p, m = divmod(n, 2)
                tab = np.where(wvalid, np.exp(rpb[n, bh])[bw_idx.ravel()]
                               .reshape(W, W), 0.0)
                ET[m * 48:m * 48 + 48,
                   (p * N_CLS + c) * NQ_MAX + dq * 48:
                   (p * N_CLS + c) * NQ_MAX + dq * 48 + 48] = tab
    return ET


def prep_weights(inputs):
    """Fold LN affines + q-scale into weights; pack for the kernel."""
    bf = ml_dtypes.bfloat16
    n1w, n1b = inputs["norm1_w"], inputs["norm1_b"]
    n2w, n2b = inputs["norm2_w"], inputs["norm2_b"]
    qkv_w, qkv_b = inputs["qkv_w"], inputs["qkv_b"]
    sc = (C // NH) ** -0.5

    Wqkv = qkv_w * n1w[None, :]
    bq = qkv_w @ n1b + qkv_b
    Wqkv[0:C] *= sc
    bq0 = bq.copy()
    bq0[0:C] *= sc

    W1 = inputs["fc1_w"] * n2w[None, :]
    b1 = inputs["fc1_w"] @ n2b + inputs["fc1_b"]

    d = {
        "wqkv_t": np.ascontiguousarray(Wqkv.T).astype(bf),       # [128, 384]
        "bqkv": np.ascontiguousarray(bq0.reshape(3, 128).T).astype(np.float32),  # [128, 3]
        "wproj_t": np.ascontiguousarray(inputs["proj_w"].T).astype(bf),
        "bproj": inputs["proj_b"].reshape(-1, 1).astype(np.float32),
        "wfc1_t": np.ascontiguousarray(W1.T).astype(bf),         # [128, 256]
        "bfc1": np.ascontiguousarray(b1.reshape(2, 128).T).astype(np.float32),
        # fc2: [128, 256] rows=khalf-cols packed: wfc2_p[:, j*128:...] = fc2_w.T[128j:128j+128, :]
        "wfc2_p": np.ascontiguousarray(
            np.concatenate([inputs["fc2_w"].T[0:128, :],
                            inputs["fc2_w"].T[128:256, :]], axis=1)).astype(bf),
        "bfc2": inputs["fc2_b"].reshape(-1, 1).astype(np.float32),
        "et": build_et_table(inputs["rpb"]).astype(bf),
    }
    return d


# --------------------------------------------------------------------------
# kernel emission
# --------------------------------------------------------------------------

def ln_block(nc, tc, sb, src_cp, dst_cp, ident, eps, tag):
    """Per-token LN over channels (gamma/beta folded downstream).
    src_cp, dst_cp: [128, NTOK] bf16 SBUF. Phase 1: transpose + stats;
    phase 2: one batched sqrt/recip; phase 3: apply + transpose back."""
    with tc.tile_pool(name=f"lnps{tag}", bufs=2, space="PSUM") as ps:
        xTs = sb.tile([128, NT * 128], BF16, tag=f"lnxT{tag}")
        mvall = sb.tile([128, NT, 2], F32, tag=f"lnmv{tag}")
        for it in range(NT):
            s = slice(it * 128, (it + 1) * 128)
            xT_ps = ps.tile([128, 128], BF16, tag=f"lnTp{tag}")
            nc.tensor.transpose(xT_ps[:], src_cp[:, s], ident[:])
            nc.vector.tensor_copy(xTs[:, s], xT_ps[:])
            stats = sb.tile([128, 6], F32, tag=f"lnst{tag}")
            nc.vector.bn_stats(stats[:], xTs[:, s])
            nc.vector.bn_aggr(mvall[:, it, :], stats[:])
        # batched rstd = 1/sqrt(var + eps)
        stdall = sb.tile([128, NT], F32, tag=f"lnsd{tag}")
        nc.scalar.activation(stdall[:], mvall[:, :, 1], AF.Sqrt, bias=eps[:])
        rstdall = sb.tile([128, NT], F32, tag=f"lnrs{tag}")
        nc.vector.reciprocal(rstdall[:], stdall[:])
        for it in range(NT):
            s = slice(it * 128, (it + 1) * 128)
            uT = sb.tile([128, 128], BF16, tag=f"lnuT{tag}")
            nc.vector.tensor_scalar(
                out=uT[:], in0=xTs[:, s], scalar1=mvall[:, it, 0:1],
                scalar2=rstdall[:, it:it + 1],
                op0=ALU.subtract, op1=ALU.mult)
            u_ps = ps.tile([128, 128], BF16, tag=f"lnup{tag}")
            nc.tensor.transpose(u_ps[:], uT[:], ident[:])
            nc.vector.tensor_copy(dst_cp[:, s], u_ps[:])


def emit_frame(nc, tc, ctx: ExitStack, debug_taps=False):
    # ---- IO ----
    x_in = nc.declare_dram_parameter("x_frame", [C, NTOK], F32, isOutput=False)
    wqkv = nc.declare_dram_parameter("wqkv_t", [C, 3 * C], BF16, isOutput=False)
    bqkv = nc.declare_dram_parameter("bqkv", [C, 3], F32, isOutput=False)
    wproj = nc.declare_dram_parameter("wproj_t", [C, C], BF16, isOutput=False)
    bproj = nc.declare_dram_parameter("bproj", [C, 1], F32, isOutput=False)
    wfc1 = nc.declare_dram_parameter("wfc1_t", [C, HID], BF16, isOutput=False)
    bfc1 = nc.declare_dram_parameter("bfc1", [C, 2], F32, isOutput=False)
    wfc2 = nc.declare_dram_parameter("wfc2_p", [C, HID], BF16, isOutput=False)
    bfc2 = nc.declare_dram_parameter("bfc2", [C, 1], F32, isOutput=False)
    et_in = nc.declare_dram_parameter("et", [96, 2 * N_CLS * NQ_MAX], BF16,
                                      isOutput=False)
    out_d = nc.declare_dram_parameter("out_frame", [C, NTOK], F32,
                                      isOutput=True)
    taps = {}
    if debug_taps:
        for nm, shp, dt in [("tap_u", [C, NTOK], BF16),
                            ("tap_qkv", [C, 3 * NTOK], BF16),
                            ("tap_k2h", [C, H * 96], BF16),
                            ("tap_vT2h", [96, H * 2 * 128], BF16),
                            ("tap_osb", [C, NTOK], BF16),
                            ("tap_dent", [C, NTOK], BF16),
                            ("tap_onorm", [C, NTOK], BF16),
                            ("tap_t", [C, NTOK], F32),
                            ("tap_z", [C, NTOK], BF16),
                            ("tap_g", [C, 2 * NTOK], BF16)]:
            taps[nm] = nc.declare_dram_parameter(nm, shp, dt, isOutput=True)

    sb = ctx.enter_context(tc.tile_pool(name="sb", bufs=3))
    big = ctx.enter_context(tc.tile_pool(name="big", bufs=1))

    # ---- loads ----
    x = big.tile([C, NTOK], F32)
    nc.sync.dma_start(x[:], x_in[:])
    w_qkv = big.tile([C, 3 * C], BF16)
    nc.sync.dma_start(w_qkv[:], wqkv[:])
    b_qkv = big.tile([C, 3], F32)
    nc.sync.dma_start(b_qkv[:], bqkv[:])
    w_proj = big.tile([C, C], BF16)
    nc.sync.dma_start(w_proj[:], wproj[:])
    b_proj = big.tile([C, 1], F32)
    nc.sync.dma_start(b_proj[:], bproj[:])
    w_fc1 = big.tile([C, HID], BF16)
    nc.sync.dma_start(w_fc1[:], wfc1[:])
    b_fc1 = big.tile([C, 2], F32)
    nc.sync.dma_start(b_fc1[:], bfc1[:])
    w_fc2 = big.tile([C, HID], BF16)
    nc.sync.dma_start(w_fc2[:], wfc2[:])
    b_fc2 = big.tile([C, 1], F32)
    nc.sync.dma_start(b_fc2[:], bfc2[:])
    et = big.tile([96, 2 * N_CLS * NQ_MAX], BF16)
    nc.sync.dma_start(et[:], et_in[:])

    ident = big.tile([128, 128], BF16)
    make_identity(nc, ident[:])
    eps_t = big.tile([128, 1], F32)
    nc.vector.memset(eps_t[:], 1e-5)
    zeros64 = big.tile([64, 512], BF16)
    nc.vector.memset(zeros64[:], 0.0)

    xb = big.tile([C, NTOK], BF16, tag="xb")
    nc.vector.tensor_copy(xb[:], x[:])

    # ---- LN1 + QKV ----
    u = big.tile([C, NTOK], BF16, tag="u")
    ln_block(nc, tc, sb, xb, u, ident, eps_t, "1")

    qkv = big.tile([128, 3 * NTOK], BF16)  # cols: [q | k | v] per NTOK
    with tc.tile_pool(name="qkvps", bufs=3, space="PSUM") as mm:
      for s in range(3):
        for ic in range(NTOK // 512 + (1 if NTOK % 512 else 0)):
            c0 = ic * 512
            cw = min(512, NTOK - c0)
            pt = mm.tile([128, 512], F32, tag="mmps")
            nc.tensor.matmul(pt[:, :cw], w_qkv[:, s * C:(s + 1) * C],
                             u[:, c0:c0 + cw], start=True, stop=True)
            nc.vector.tensor_scalar_add(
                out=qkv[:, s * NTOK + c0: s * NTOK + c0 + cw],
                in0=pt[:, :cw], scalar1=b_qkv[:, s:s + 1])

    q_cp = qkv[:, 0:NTOK]
    k_cp = qkv[:, NTOK:2 * NTOK]
    v_cp = qkv[:, 2 * NTOK:3 * NTOK]

    # ---- k2h: [128=(p,m,c), H*96] ----
    k2h = big.tile([128, H * 96], BF16, tag="k2h")
    nc.vector.memset(k2h[:], 0.0)
    for p in range(2):
        for m in range(2):
            rows = slice(p * 64 + m * 32, p * 64 + m * 32 + 32)
            dst = k2h[rows, :].rearrange("c (r g) -> c r g", g=96)[
                :, :, m * 48:m * 48 + W]
            src = k_cp[rows, :].rearrange("c (r w) -> c r w", r=H)
            nc.vector.tensor_copy(dst, src)

    # ---- v2h_pre: [128 rows: 66*p + (m*32..)|64,65-ones, H*96] ----
    v2h_pre = big.tile([128, 2 * H * 96], BF16, tag="v2h")   # pair p at cols p*H*96
    nc.vector.memset(v2h_pre[:], 0.0)
    for p in range(2):
        po = p * H * 96
        for m in range(2):
            rows = slice(m * 32, m * 32 + 32)
            dst = v2h_pre[rows, po:po + H * 96].rearrange(
                "c (r g) -> c r g", g=96)[:, :, m * 48:m * 48 + W]
            src = v_cp[p * 64 + m * 32:p * 64 + m * 32 + 32, :] \
                .rearrange("c (r w) -> c r w", r=H)
            nc.vector.tensor_copy(dst, src)
            # ones rows 64-95 (m0) / 96-127 (m1): after transpose these
            # become stationary cols 64-95/96-127, so the AV matmul writes
            # the per-head denominator REPLICATED over 32 rows -- the
            # partition broadcast happens inside the matmul for free.
            ones_dst = v2h_pre[64 + 32 * m: 96 + 32 * m, po:po + H * 96] \
                .rearrange("c (r g) -> c r g", g=96)[:, :, m * 48:m * 48 + W]
            nc.vector.memset(ones_dst, 1.0)

    # transpose to vT2h [96, (r, p)*128]; batch 4 transposes per PSUM bank
    vT2h = big.tile([96, H * 2 * 128], BF16, tag="vT2h")
    with tc.tile_pool(name="vTpool", bufs=2, space="PSUM") as psv:
        for g in range(H * 2 // 4):
            tps = psv.tile([96, 512], BF16, tag="vTps")
            for k in range(4):
                idx = 4 * g + k
                r, p = divmod(idx, 2)
                nc.tensor.matmul(
                    tps[:, k * 128:(k + 1) * 128],
                    v2h_pre[:, p * H * 96 + r * 96:p * H * 96 + (r + 1) * 96],
                    ident[:], is_transpose=True,
                    start=(k == 0), stop=(k == 3), skip_group_check=True)
            dst = vT2h[:, g * 512:(g + 1) * 512]
            if g % 2 == 0:
                nc.scalar.copy(dst, tps[:])
            else:
                nc.vector.tensor_copy(dst, tps[:])


    if debug_taps:
        nc.sync.dma_start(taps["tap_k2h"][:], k2h[:])
        nc.sync.dma_start(taps["tap_vT2h"][:], vT2h[:])
    # ---- attention ----
    osb = big.tile([C, NTOK], BF16)     # unnormalized O rows (c-order)
    den_all = big.tile([C, NTOK], BF16, tag="dent")  # per-head denom, bcast
    with tc.tile_pool(name="attps", bufs=3, space="PSUM") as ps, \
         tc.tile_pool(name="attpsO", bufs=1, space="PSUM") as psO:
        for p in range(2):
            ot = psO.tile([128, NTOK], F32, tag="ot")
            # claim + zero all banks of ot via K=64 zero matmuls (stay in
            # 64-row tile mode, matching the scores matmuls)
            for j in range(0, NTOK, 512):
                jw = min(512, NTOK - j)
                nc.tensor.matmul(ot[:, j:j + jw], zeros64[:, 0:128],
                                 zeros64[:, 0:jw], start=True, stop=True,
                                 skip_group_check=True)
            for r in range(H):
                qs, qn = qwin(r)
                nq = 48 * qn
                cls = et_cls(r)
                st = ps.tile([96, NQ_MAX], F32, tag="st")
                nc.tensor.matmul(
                    st[:, :nq],
                    k2h[p * 64:(p + 1) * 64, r * 96:(r + 1) * 96],
                    q_cp[p * 64:(p + 1) * 64, 48 * qs:48 * qs + nq],
                    start=True, stop=True)
                pexp = sb.tile([96, NQ_MAX], BF16, tag="pexp")
                nc.scalar.activation(pexp[:, :nq], st[:, :nq], AF.Exp)
                pmul = sb.tile([96, NQ_MAX], BF16, tag="pmul")
                nc.vector.tensor_mul(
                    pmul[:, :nq], pexp[:, :nq],
                    et[:, (p * N_CLS + cls) * NQ_MAX:
                          (p * N_CLS + cls) * NQ_MAX + nq])
                # AV accumulate, split at PSUM bank boundaries (512 f32)
                c0 = 48 * qs
                b0 = (c0 // 512 + 1) * 512
                if b0 < c0 + nq:
                    splits = [(c0, b0 - c0), (b0, c0 + nq - b0)]
                else:
                    splits = [(c0, nq)]
                for (cs_, nw) in splits:
                    nc.tensor.matmul(
                        ot[:, cs_:cs_ + nw],
                        vT2h[:, (r * 2 + p) * 128:(r * 2 + p) * 128 + 128],
                        pmul[:, cs_ - c0:cs_ - c0 + nw],
                        start=False, stop=(r == H - 1), skip_group_check=True)
            # drains: c rows on DVE, replicated denom rows 64-127 on ScalarE
            nc.vector.tensor_copy(osb[64 * p:64 * p + 64, :], ot[0:64, :])
            nc.scalar.copy(den_all[64 * p:64 * p + 64, :], ot[64:128, :])

    if debug_taps:
        nc.sync.dma_start(taps["tap_osb"][:], osb[:])
        nc.sync.dma_start(taps["tap_dent"][:], den_all[:])
    recip_b = big.tile([C, NTOK], BF16, tag="v2h")
    with nc.allow_low_precision(reason="denominators are O(1..50); bf16 "
                                "recip error ~0.4% well under tolerance"):
        nc.vector.reciprocal(recip_b[:], den_all[:])
    onorm = big.tile([C, NTOK], BF16)
    nc.vector.tensor_mul(onorm[:], osb[:], recip_b[:])

    if debug_taps:
        nc.sync.dma_start(taps["tap_onorm"][:], onorm[:])
    # ---- proj + residual -> t (f32), tb (bf16) ----
    mm = ctx.enter_context(tc.tile_pool(name="tailps", bufs=3, space="PSUM"))
    t_res = big.tile([C, NTOK], F32)
    for ic in range(0, NTOK, 512):
        cw = min(512, NTOK - ic)
        pt = mm.tile([128, 512], F32, tag="mmps")
        nc.tensor.matmul(pt[:, :cw], w_proj[:], onorm[:, ic:ic + cw],
                         start=True, stop=True)
        nc.vector.scalar_tensor_tensor(
            out=t_res[:, ic:ic + cw], in0=pt[:, :cw],
            scalar=b_proj[:, 0:1], in1=x[:, ic:ic + cw],
            op0=ALU.add, op1=ALU.add)
    tb = big.tile([C, NTOK], BF16, tag="xb")
    nc.vector.tensor_copy(tb[:], t_res[:])

    if debug_taps:
        nc.sync.dma_start(taps["tap_t"][:], t_res[:])
    # ---- LN2 ----
    z = big.tile([C, NTOK], BF16, tag="u")
    ln_block(nc, tc, sb, tb, z, ident, eps_t, "2")

    # ---- MLP ----
    g = big.tile([128, 2 * NTOK], BF16, tag="k2h")   # reuses k2h slot
    for j in range(2):
        for ic in range(0, NTOK, 512):
            cw = min(512, NTOK - ic)
            pt = mm.tile([128, 512], F32, tag="mmps")
            nc.tensor.matmul(pt[:, :cw], w_fc1[:, j * 128:(j + 1) * 128],
                             z[:, ic:ic + cw], start=True, stop=True)
            nc.scalar.activation(
                g[:, j * NTOK + ic: j * NTOK + ic + cw], pt[:, :cw],
                AF.Gelu, bias=b_fc1[:, j:j + 1])

    if debug_taps:
        nc.sync.dma_start(taps["tap_z"][:], z[:])
        nc.sync.dma_start(taps["tap_g"][:], g[:])
    out_t = big.tile([C, NTOK], F32, tag="dent")
    for ic in range(0, NTOK, 512):
        cw = min(512, NTOK - ic)
        pt = mm.tile([128, 512], F32, tag="mmps")
        for j in range(2):
            nc.tensor.matmul(pt[:, :cw], w_fc2[:, j * C:(j + 1) * C],
                             g[:, j * NTOK + ic: j * NTOK + ic + cw],
                             start=(j == 0), stop=(j == 1))
        nc.vector.scalar_tensor_tensor(
            out=out_t[:, ic:ic + cw], in0=pt[:, :cw],
            scalar=b_fc2[:, 0:1], in1=t_res[:, ic:ic + cw],
            op0=ALU.add, op1=ALU.add)

    nc.sync.dma_start(out_d[:], out_t[:])


# --------------------------------------------------------------------------
# SPMD entry point: full inputs -> full output on 8 NeuronCores
# --------------------------------------------------------------------------
import concourse.tile as _tile
import concourse.bacc as _bacc
from concourse.bass_utils import run_bass_kernel_spmd as _run_spmd

_CACHE = {}


def _get_nc():
    if "nc" not in _CACHE:
        nc = _bacc.Bacc("TRN2", target_bir_lowering=False, debug=False,
                        num_devices=8)
        with _tile.TileContext(nc) as tc:
            with ExitStack() as ctx:
                emit_frame(nc, tc, ctx)
        nc.compile()
        _CACHE["nc"] = nc
    return _CACHE["nc"]


def kernel(**inputs):
    inputs = {k: np.asarray(v) for k, v in inputs.items()}
    x = inputs["x"]
    B, Cc, D, Hh, Ww = x.shape          # (2, 128, 4, 48, 48)
    assert (B, Cc, D, Hh, Ww) == (2, 128, 4, 48, 48)
    wd = prep_weights(inputs)
    nc = _get_nc()

    in_maps = []
    for core in range(8):
        b, dd = divmod(core, D)
        frame = np.ascontiguousarray(x[b, :, dd]).reshape(C, NTOK)
        m = {"x_frame": frame.astype(np.float32)}
        m.update(wd)
        in_maps.append(m)

    res = _run_spmd(nc, in_maps, list(range(8)))
    out = np.empty((B, Cc, D, Hh, Ww), np.float32)
    for core in range(8):
        b, dd = divmod(core, D)
        out[b, :, dd] = res.results[core]["out_frame"].reshape(C, Hh, Ww)
    return out

